# revision 4
# baseline (speedup 1.0000x reference)
"""TRN2 Bass kernel for the ConceptualMambaBlock problem.

Math (reference):
    x: [B=4, T=96, N=512, H=128] f32
    expanded = x @ W_exp.T + b_exp            # [B,T,N,2H]
    primary, gating = split(expanded, 2, -1)
    s_t = 0.9*s_{t-1} + 0.1*gating_t          # EMA along T
    out = (primary * sigmoid(s)) @ W_con.T + b_con

Strategy:
  - Shard (B x N/2) over 8 cores: core c -> batch c//2, node half c%2.
  - Host pre-transposes each core's x shard to [H, N_local, T] so the
    contraction dim H lands on SBUF partitions with fully-contiguous DMA;
    no on-chip transposes anywhere.
  - Per 4-node block (tok = 4*96 = 384 columns, t fastest):
      mm1 (fp32r, full PE rate) -> PSUM gating/primary [o=128, tok]
      gating bias via K=1 accumulate-matmul (weights/bias pre-scaled by 0.1)
      EMA via DVE tensor_tensor_scan: state = mask*state + g  (mask has 0.0
      at each t=0 column, so the 4 node-segments reset exactly)
      sigmoid on ACT; gate-mul + primary bias in one DVE op;
      mm2 (fp32r); output bias via ACT Identity; DMA out.
  - Output returned as [H, N_local, T] per core; host transposes back.
"""

import numpy as np

import concourse.bacc as bacc
import concourse.bass as bass  # noqa: F401  (engine types referenced via nc)
import concourse.mybir as mybir
import concourse.tile as tile
from concourse.bass_utils import run_bass_kernel_spmd

F32 = mybir.dt.float32
F32R = mybir.dt.float32r
AF = mybir.ActivationFunctionType
ALU = mybir.AluOpType

B, T, N, H = 4, 96, 512, 128
NCORES = 8
NLOC = N // 2          # 256 nodes per core
NB = 4                 # nodes per block
TOK = NB * T           # 384 columns per block
NBLK = NLOC // NB      # 64 blocks per core

_NC_CACHE = None


def _build():
    nc = bacc.Bacc()

    xt_h = nc.dram_tensor("xt", [H, NBLK, TOK], F32R, kind="ExternalInput")
    w1p_h = nc.dram_tensor("w1p", [H, H], F32R, kind="ExternalInput")
    w1g_h = nc.dram_tensor("w1g", [H, H], F32R, kind="ExternalInput")
    w2_h = nc.dram_tensor("w2", [H, H], F32R, kind="ExternalInput")
    b1g_h = nc.dram_tensor("b1g", [1, H], F32R, kind="ExternalInput")
    b1p_h = nc.dram_tensor("b1p", [H, 1], F32, kind="ExternalInput")
    b2_h = nc.dram_tensor("b2", [H, 1], F32, kind="ExternalInput")
    ones_h = nc.dram_tensor("ones", [1, TOK], F32R, kind="ExternalInput")
    out_h = nc.dram_tensor("out", [H, NBLK, TOK], F32, kind="ExternalOutput")

    with tile.TileContext(nc) as tc:
        with (
            tc.tile_pool(name="consts", bufs=1) as cp,
            tc.tile_pool(name="io", bufs=4) as io,
            tc.tile_pool(name="ps", bufs=2, space="PSUM") as ps,
        ):
            w1p_sb = cp.tile([H, H], F32R, tag="w1p")
            nc.gpsimd.dma_start(out=w1p_sb[:], in_=w1p_h[:, :])
            w1g_sb = cp.tile([H, H], F32R, tag="w1g")
            nc.gpsimd.dma_start(out=w1g_sb[:], in_=w1g_h[:, :])
            w2_sb = cp.tile([H, H], F32R, tag="w2")
            nc.gpsimd.dma_start(out=w2_sb[:], in_=w2_h[:, :])
            b1g_sb = cp.tile([1, H], F32R, tag="b1g")
            nc.gpsimd.dma_start(out=b1g_sb[:], in_=b1g_h[:, :])
            b1p_sb = cp.tile([H, 1], F32, tag="b1p")
            nc.gpsimd.dma_start(out=b1p_sb[:], in_=b1p_h[:, :])
            b2_sb = cp.tile([H, 1], F32, tag="b2")
            nc.gpsimd.dma_start(out=b2_sb[:], in_=b2_h[:, :])

            ones_sb = cp.tile([1, TOK], F32R, tag="ones")
            nc.gpsimd.dma_start(out=ones_sb[:], in_=ones_h[:, :])
            mask_sb = cp.tile([H, NB, T], F32, tag="mask")
            nc.gpsimd.memset(mask_sb[:], 0.9)
            nc.gpsimd.memset(mask_sb[:, :, 0:1], 0.0)
            mask2d = mask_sb[:].rearrange("p a b -> p (a b)")

            for nb in range(NBLK):
                xt = io.tile([H, TOK], F32R, tag="xt")
                nc.sync.dma_start(out=xt[:], in_=xt_h[:, nb, :])

                pg = ps.tile([H, TOK], F32, tag="pg")
                nc.tensor.matmul(
                    pg[:], lhsT=b1g_sb[:], rhs=ones_sb[:], start=True, stop=False
                )
                nc.tensor.matmul(
                    pg[:], lhsT=w1g_sb[:], rhs=xt[:], start=False, stop=True
                )
                pp = ps.tile([H, TOK], F32, tag="pp")
                nc.tensor.matmul(pp[:], lhsT=w1p_sb[:], rhs=xt[:], start=True, stop=True)

                s = io.tile([H, TOK], F32, tag="s")
                nc.vector.tensor_tensor_scan(
                    out=s[:],
                    data0=mask2d,
                    data1=pg[:],
                    initial=0.0,
                    op0=ALU.mult,
                    op1=ALU.add,
                )
                sg = io.tile([H, TOK], F32, tag="sg")
                nc.scalar.activation(sg[:], s[:], AF.Sigmoid)

                y = io.tile([H, TOK], F32R, tag="y")
                nc.vector.scalar_tensor_tensor(
                    out=y[:],
                    in0=pp[:],
                    scalar=b1p_sb[:],
                    in1=sg[:],
                    op0=ALU.add,
                    op1=ALU.mult,
                )

                po = ps.tile([H, TOK], F32, tag="po")
                nc.tensor.matmul(po[:], lhsT=w2_sb[:], rhs=y[:], start=True, stop=True)

                ob = io.tile([H, TOK], F32, tag="ob")
                nc.scalar.activation(ob[:], po[:], AF.Identity, bias=b2_sb[:], scale=1.0)
                nc.sync.dma_start(out=out_h[:, nb, :], in_=ob[:])

    nc.finalize()
    return nc


def _get_nc():
    global _NC_CACHE
    if _NC_CACHE is None:
        _NC_CACHE = _build()
    return _NC_CACHE


def _in_maps(x, W_exp, b_exp, W_con, b_con):
    w1p = np.ascontiguousarray(W_exp[:H, :].T, dtype=np.float32)
    w1g = np.ascontiguousarray((0.1 * W_exp[H:, :]).T, dtype=np.float32)
    w2 = np.ascontiguousarray(W_con.T, dtype=np.float32)
    b1g = np.ascontiguousarray((0.1 * b_exp[H:]).reshape(1, H), dtype=np.float32)
    b1p = np.ascontiguousarray(b_exp[:H].reshape(H, 1), dtype=np.float32)
    b2 = np.ascontiguousarray(b_con.reshape(H, 1), dtype=np.float32)

    maps = []
    for c in range(NCORES):
        bb, nh = c // 2, c % 2
        xs = x[bb, :, nh * NLOC : (nh + 1) * NLOC, :]  # [T, NLOC, H]
        xT = np.ascontiguousarray(xs.transpose(2, 1, 0)).reshape(H, NBLK, TOK)
        maps.append(
            {
                "xt": xT,
                "w1p": w1p,
                "w1g": w1g,
                "w2": w2,
                "b1g": b1g,
                "b1p": b1p,
                "b2": b2,
                "ones": np.ones((1, TOK), dtype=np.float32),
            }
        )
    return maps


def run_spmd(x, W_exp, b_exp, W_con, b_con, **spmd_kwargs):
    """Run the 8-core kernel; returns (full_output, BassKernelResults)."""
    maps = _in_maps(x, W_exp, b_exp, W_con, b_con)
    res = run_bass_kernel_spmd(
        _get_nc(), maps, core_ids=list(range(NCORES)), **spmd_kwargs
    )
    out = np.empty((B, T, N, H), dtype=np.float32)
    for c in range(NCORES):
        bb, nh = c // 2, c % 2
        oT = res.results[c]["out"].reshape(H, NLOC, T)
        out[bb, :, nh * NLOC : (nh + 1) * NLOC, :] = oT.transpose(2, 1, 0)
    return out, res


def kernel(spatial_temporal_representation, W_exp, b_exp, W_con, b_con):
    out, _ = run_spmd(
        np.asarray(spatial_temporal_representation, dtype=np.float32),
        np.asarray(W_exp, dtype=np.float32),
        np.asarray(b_exp, dtype=np.float32),
        np.asarray(W_con, dtype=np.float32),
        np.asarray(b_con, dtype=np.float32),
    )
    return out


# revision 5
# speedup vs baseline: 1.0882x; 1.0882x over previous
"""TRN2 Bass kernel for the ConceptualMambaBlock problem.

Math (reference):
    x: [B=4, T=96, N=512, H=128] f32
    expanded = x @ W_exp.T + b_exp            # [B,T,N,2H]
    primary, gating = split(expanded, 2, -1)
    s_t = 0.9*s_{t-1} + 0.1*gating_t          # EMA along T
    out = (primary * sigmoid(s)) @ W_con.T + b_con

Strategy:
  - Shard (B x N/2) over 8 cores: core c -> batch c//2, node half c%2.
  - Host pre-transposes each core's x shard to [H, N_local, T] so the
    contraction dim H lands on SBUF partitions with fully-contiguous DMA;
    no on-chip transposes anywhere.
  - Per 4-node block (tok = 4*96 = 384 columns, t fastest):
      mm1 (fp32r, full PE rate) -> PSUM gating/primary [o=128, tok]
      gating bias via K=1 accumulate-matmul (weights/bias pre-scaled by 0.1)
      EMA via DVE tensor_tensor_scan: state = mask*state + g  (mask has 0.0
      at each t=0 column, so the 4 node-segments reset exactly)
      sigmoid on ACT; gate-mul + primary bias in one DVE op;
      mm2 (fp32r); output bias via ACT Identity; DMA out.
  - DMA is grouped: one load / one store covers GRP consecutive blocks.
  - Output returned as [H, N_local, T] per core; host transposes back.
"""

import numpy as np

import concourse.bacc as bacc
import concourse.bass as bass  # noqa: F401  (engine types referenced via nc)
import concourse.mybir as mybir
import concourse.tile as tile
from concourse.bass_utils import run_bass_kernel_spmd

F32 = mybir.dt.float32
F32R = mybir.dt.float32r
AF = mybir.ActivationFunctionType
ALU = mybir.AluOpType

B, T, N, H = 4, 96, 512, 128
NCORES = 8
NLOC = N // 2          # 256 nodes per core
NB = 4                 # nodes per block
TOK = NB * T           # 384 columns per block
NBLK = NLOC // NB      # 64 blocks per core
GRP = 4                # blocks per DMA group
NGRP = NBLK // GRP

_NC_CACHE = None


def _build():
    nc = bacc.Bacc()

    xt_h = nc.dram_tensor("xt", [H, NBLK, TOK], F32R, kind="ExternalInput")
    w1p_h = nc.dram_tensor("w1p", [H, H], F32R, kind="ExternalInput")
    w1g_h = nc.dram_tensor("w1g", [H, H], F32R, kind="ExternalInput")
    w2_h = nc.dram_tensor("w2", [H, H], F32R, kind="ExternalInput")
    b1g_h = nc.dram_tensor("b1g", [1, H], F32R, kind="ExternalInput")
    b1p_h = nc.dram_tensor("b1p", [H, 1], F32, kind="ExternalInput")
    b2_h = nc.dram_tensor("b2", [H, 1], F32, kind="ExternalInput")
    ones_h = nc.dram_tensor("ones", [1, TOK], F32R, kind="ExternalInput")
    out_h = nc.dram_tensor("out", [H, NBLK, TOK], F32, kind="ExternalOutput")

    with tile.TileContext(nc) as tc:
        with (
            tc.tile_pool(name="consts", bufs=1) as cp,
            tc.tile_pool(name="io", bufs=3) as io,
            tc.tile_pool(name="mid", bufs=6) as mid,
            tc.tile_pool(name="ps", bufs=2, space="PSUM") as ps,
        ):
            w1p_sb = cp.tile([H, H], F32R, tag="w1p")
            nc.gpsimd.dma_start(out=w1p_sb[:], in_=w1p_h[:, :])
            w1g_sb = cp.tile([H, H], F32R, tag="w1g")
            nc.gpsimd.dma_start(out=w1g_sb[:], in_=w1g_h[:, :])
            w2_sb = cp.tile([H, H], F32R, tag="w2")
            nc.gpsimd.dma_start(out=w2_sb[:], in_=w2_h[:, :])
            b1g_sb = cp.tile([1, H], F32R, tag="b1g")
            nc.gpsimd.dma_start(out=b1g_sb[:], in_=b1g_h[:, :])
            b1p_sb = cp.tile([H, 1], F32, tag="b1p")
            nc.gpsimd.dma_start(out=b1p_sb[:], in_=b1p_h[:, :])
            b2_sb = cp.tile([H, 1], F32, tag="b2")
            nc.gpsimd.dma_start(out=b2_sb[:], in_=b2_h[:, :])
            ones_sb = cp.tile([1, TOK], F32R, tag="ones")
            nc.gpsimd.dma_start(out=ones_sb[:], in_=ones_h[:, :])

            mask_sb = cp.tile([H, NB, T], F32, tag="mask")
            nc.gpsimd.memset(mask_sb[:], 0.9)
            nc.gpsimd.memset(mask_sb[:, :, 0:1], 0.0)
            mask2d = mask_sb[:].rearrange("p a b -> p (a b)")

            for g in range(NGRP):
                xt4 = io.tile([H, GRP, TOK], F32R, tag="xt")
                nc.sync.dma_start(out=xt4[:], in_=xt_h[:, g * GRP : (g + 1) * GRP, :])
                ob4 = io.tile([H, GRP, TOK], F32, tag="ob")

                for j in range(GRP):
                    xt = xt4[:, j, :]
                    pg = ps.tile([H, TOK], F32, tag="pg")
                    nc.tensor.matmul(
                        pg[:], lhsT=b1g_sb[:], rhs=ones_sb[:], start=True, stop=False
                    )
                    nc.tensor.matmul(
                        pg[:], lhsT=w1g_sb[:], rhs=xt, start=False, stop=True
                    )
                    pp = ps.tile([H, TOK], F32, tag="pp")
                    nc.tensor.matmul(pp[:], lhsT=w1p_sb[:], rhs=xt, start=True, stop=True)

                    s = mid.tile([H, TOK], F32, tag="s")
                    nc.vector.tensor_tensor_scan(
                        out=s[:],
                        data0=mask2d,
                        data1=pg[:],
                        initial=0.0,
                        op0=ALU.mult,
                        op1=ALU.add,
                    )
                    sg = mid.tile([H, TOK], F32, tag="sg")
                    nc.scalar.activation(sg[:], s[:], AF.Sigmoid)

                    y = mid.tile([H, TOK], F32R, tag="y")
                    nc.vector.scalar_tensor_tensor(
                        out=y[:],
                        in0=pp[:],
                        scalar=b1p_sb[:],
                        in1=sg[:],
                        op0=ALU.add,
                        op1=ALU.mult,
                    )

                    po = ps.tile([H, TOK], F32, tag="po")
                    nc.tensor.matmul(po[:], lhsT=w2_sb[:], rhs=y[:], start=True, stop=True)

                    nc.scalar.activation(
                        ob4[:, j, :], po[:], AF.Identity, bias=b2_sb[:], scale=1.0
                    )

                nc.gpsimd.dma_start(
                    out=out_h[:, g * GRP : (g + 1) * GRP, :], in_=ob4[:]
                )

    nc.finalize()
    return nc


def _get_nc():
    global _NC_CACHE
    if _NC_CACHE is None:
        _NC_CACHE = _build()
    return _NC_CACHE


def _in_maps(x, W_exp, b_exp, W_con, b_con):
    w1p = np.ascontiguousarray(W_exp[:H, :].T, dtype=np.float32)
    w1g = np.ascontiguousarray((0.1 * W_exp[H:, :]).T, dtype=np.float32)
    w2 = np.ascontiguousarray(W_con.T, dtype=np.float32)
    b1g = np.ascontiguousarray((0.1 * b_exp[H:]).reshape(1, H), dtype=np.float32)
    b1p = np.ascontiguousarray(b_exp[:H].reshape(H, 1), dtype=np.float32)
    b2 = np.ascontiguousarray(b_con.reshape(H, 1), dtype=np.float32)

    maps = []
    for c in range(NCORES):
        bb, nh = c // 2, c % 2
        xs = x[bb, :, nh * NLOC : (nh + 1) * NLOC, :]  # [T, NLOC, H]
        xT = np.ascontiguousarray(xs.transpose(2, 1, 0)).reshape(H, NBLK, TOK)
        maps.append(
            {
                "xt": xT,
                "w1p": w1p,
                "w1g": w1g,
                "w2": w2,
                "b1g": b1g,
                "b1p": b1p,
                "b2": b2,
                "ones": np.ones((1, TOK), dtype=np.float32),
            }
        )
    return maps


def run_spmd(x, W_exp, b_exp, W_con, b_con, **spmd_kwargs):
    """Run the 8-core kernel; returns (full_output, BassKernelResults)."""
    maps = _in_maps(x, W_exp, b_exp, W_con, b_con)
    res = run_bass_kernel_spmd(
        _get_nc(), maps, core_ids=list(range(NCORES)), **spmd_kwargs
    )
    out = np.empty((B, T, N, H), dtype=np.float32)
    for c in range(NCORES):
        bb, nh = c // 2, c % 2
        oT = res.results[c]["out"].reshape(H, NLOC, T)
        out[bb, :, nh * NLOC : (nh + 1) * NLOC, :] = oT.transpose(2, 1, 0)
    return out, res


def kernel(spatial_temporal_representation, W_exp, b_exp, W_con, b_con):
    out, _ = run_spmd(
        np.asarray(spatial_temporal_representation, dtype=np.float32),
        np.asarray(W_exp, dtype=np.float32),
        np.asarray(b_exp, dtype=np.float32),
        np.asarray(W_con, dtype=np.float32),
        np.asarray(b_con, dtype=np.float32),
    )
    return out
